# revision 1
# baseline (speedup 1.0000x reference)
# MLA (multi-head latent attention) forward on 8 Trainium2 NeuronCores.
#
# Sharding: data-parallel over batch (2) x tensor-parallel over heads (4
# heads/core). Core c handles batch c//4 and heads 4*(c%4)..+4. The small
# latent a-projections are replicated inside each batch group; o_proj is
# computed as per-core partials over the local heads' rows and reduced on
# the host during unsharding.
#
# On-device layout is feature-major ([feature_partition, token_free]) so
# chained matmuls never transpose activations. All matmuls run as float32r
# (TF32-like, 1 cycle/row at N>=256). LayerNorm is folded: gamma into the
# b-projection weights, beta as a per-partition constant add, the per-token
# mean via an extra column of the a-projection weights, and the variance via
# an ACT square pass + ones-column matmul reduction. Attention is computed
# k-major (scores.T = [k_tok, q_tok]) which feeds P.T directly into the
# PV matmul with no transposes; the softmax denominator is a ones-column
# matmul over P.T and the causal mask is a 0/1 triangle multiply after exp.
import sys

sys.path.insert(0, "/opt/trn_rl_repo")

import numpy as np

H = 16
DN = 128
DR = 64
DV = 128
QL = 1536
KL = 512
HID = 2048
B = 2
S = 1024
NCORES = 8
TP = 4          # head groups (cores per batch)
HPC = H // TP   # heads per core
EPS = 1e-5
SCALE = 1.0 / float(np.sqrt(DN + DR))

KQ = QL // 128      # 12 q-latent feature tiles
KKV = KL // 128     # 4 kv-latent feature tiles
KX = HID // 128     # 16 x feature tiles
NS = S // 128       # 8 token tiles
MQB = HPC * (DN + DR) // 128   # 6 q_b output tiles (4 nope + 2 rope pairs)
MO = HID // 128     # 16 o_proj output tiles

TRACE = False
USE_BF16 = False
_COMPILED = None


def _build():
    import concourse.mybir as mybir
    import concourse.tile as tile
    from concourse import bacc

    F32 = mybir.dt.float32
    F32R = mybir.dt.float32r
    WDT = mybir.dt.bfloat16 if USE_BF16 else F32R
    AF = mybir.ActivationFunctionType
    from concourse.alu_op_type import AluOpType

    nc = bacc.Bacc("TRN2", target_bir_lowering=False, debug=False)

    # ---- DRAM tensors (per-core inputs; same shapes on every core) ----
    xT_d = nc.dram_tensor("xT", [KX, 128, S], WDT, kind="ExternalInput")
    wqa_d = nc.dram_tensor("wqa", [KQ, 128, KX, 128], WDT, kind="ExternalInput")
    wkva_d = nc.dram_tensor("wkva", [5, 128, KX, 128], WDT, kind="ExternalInput")
    wqb_d = nc.dram_tensor("wqb", [MQB, 128, KQ, 128], WDT, kind="ExternalInput")
    wkbk_d = nc.dram_tensor("wkbk", [HPC, 128, KKV, 128], WDT, kind="ExternalInput")
    wkbv_d = nc.dram_tensor("wkbv", [128, KKV, HPC * DV], WDT, kind="ExternalInput")
    wo_d = nc.dram_tensor("wo", [MO, 128, HPC, 128], WDT, kind="ExternalInput")
    c128_d = nc.dram_tensor("c128", [128, S], F32, kind="ExternalInput")
    s128_d = nc.dram_tensor("s128", [128, S], F32, kind="ExternalInput")
    tri_d = nc.dram_tensor("tri", [128, 128], F32, kind="ExternalInput")
    ones_d = nc.dram_tensor("ones", [128, 1], WDT, kind="ExternalInput")
    brow_d = nc.dram_tensor("brow", [1, 128], WDT, kind="ExternalInput")
    pswap_d = nc.dram_tensor("pswap", [128, 128], WDT, kind="ExternalInput")
    pdup_d = nc.dram_tensor("pdup", [64, 128], WDT, kind="ExternalInput")
    pdupsw_d = nc.dram_tensor("pdupsw", [64, 128], WDT, kind="ExternalInput")
    cq_d = nc.dram_tensor("cq", [128, MQB], F32, kind="ExternalInput")
    ckv_d = nc.dram_tensor("ckv", [128, HPC], F32, kind="ExternalInput")
    bvc_d = nc.dram_tensor("bvc", [128, HPC], F32, kind="ExternalInput")
    o_d = nc.dram_tensor("o_part", [HID, S], F32, kind="ExternalOutput")

    CH = (slice(0, 512), slice(512, 1024))  # 512-wide token chunks

    with tile.TileContext(nc) as tc:
        with (
            tc.tile_pool(name="const", bufs=1) as constp,
            tc.tile_pool(name="xt", bufs=1) as xtp,
            tc.tile_pool(name="z", bufs=1) as zp,
            tc.tile_pool(name="wpan", bufs=3) as wp,
            tc.tile_pool(name="sq", bufs=2) as sqp,
            tc.tile_pool(name="rows", bufs=5) as rowp,
            tc.tile_pool(name="lnb", bufs=3) as lnbp,
            tc.tile_pool(name="act", bufs=1) as actp,
            tc.tile_pool(name="pt", bufs=3) as ptp,
                        tc.tile_pool(name="mm", bufs=3, space="PSUM") as mmp,
            tc.tile_pool(name="arow", bufs=1, space="PSUM") as arp,
            tc.tile_pool(name="num", bufs=2, space="PSUM") as nump,
            tc.tile_pool(name="den", bufs=2, space="PSUM") as denp,
        ):
            # ---- constants ----
            tri = constp.tile([128, 128], F32)
            nc.gpsimd.dma_start(out=tri, in_=tri_d.ap())
            ones = constp.tile([128, 1], WDT)
            nc.gpsimd.dma_start(out=ones, in_=ones_d.ap())
            brow = constp.tile([1, 128], WDT)
            nc.gpsimd.dma_start(out=brow, in_=brow_d.ap())
            pswap = constp.tile([128, 128], WDT)
            nc.gpsimd.dma_start(out=pswap, in_=pswap_d.ap())
            pdup = constp.tile([64, 128], WDT)
            nc.gpsimd.dma_start(out=pdup, in_=pdup_d.ap())
            pdupsw = constp.tile([64, 128], WDT)
            nc.gpsimd.dma_start(out=pdupsw, in_=pdupsw_d.ap())
            cq = constp.tile([128, MQB], F32)
            nc.gpsimd.dma_start(out=cq, in_=cq_d.ap())
            ckv = constp.tile([128, HPC], F32)
            nc.gpsimd.dma_start(out=ckv, in_=ckv_d.ap())
            bvc = constp.tile([128, HPC], F32)
            nc.gpsimd.dma_start(out=bvc, in_=bvc_d.ap())
            eps_t = constp.tile([1, 1], F32)
            nc.vector.memset(eps_t, EPS)

            # persistent (full-width) attention operands
            knope = [actp.tile([128, S], WDT, tag=f"kn{h}", name=f"kn{h}")
                     for h in range(HPC)]
            vt = [actp.tile([128, HPC * DV], WDT, tag=f"v{st}", name=f"v{st}")
                  for st in range(NS)]
            krope = actp.tile([128, S], WDT, tag="krope")
            qfull = [actp.tile([128, S], WDT, tag=f"qf{m}", name=f"qf{m}")
                     for m in range(MQB)]
            attn = [actp.tile([128, S], WDT, tag=f"at{h}", name=f"at{h}")
                    for h in range(HPC)]

            # ---- per-token-chunk pipeline: a-proj -> LN -> b-proj -> rope ----
            for c in range(2):
                ch = CH[c]
                pan0 = wp.tile([128, KX, 128], WDT, tag="w", name=f"pan0_{c}")
                nc.sync.dma_start(out=pan0, in_=wkva_d.ap()[0])
                xt = []
                for k in range(KX):
                    t = xtp.tile([128, 512], WDT, tag=f"xt{k}", name=f"xt{k}_{c}")
                    eng = nc.scalar if k % 2 == 0 else nc.sync
                    eng.dma_start(out=t, in_=xT_d.ap()[k][:, ch])
                    xt.append(t)

                c_t = sqp.tile([128, 512], F32, tag="cs", bufs=2, name=f"c{c}")
                nc.gpsimd.dma_start(out=c_t, in_=c128_d.ap()[:, ch])
                s_t = sqp.tile([128, 512], F32, tag="cs", bufs=2, name=f"s{c}")
                nc.gpsimd.dma_start(out=s_t, in_=s128_d.ap()[:, ch])

                def aproj(nmt, w_dram, sq_ps, nsq, zs, pre=None):
                    for m in range(nmt):
                        if pre is not None and m == 0:
                            pan = pre
                        else:
                            pan = wp.tile([128, KX, 128], WDT, tag="w",
                                          name=f"p{w_dram.name}{m}_{c}")
                            nc.sync.dma_start(out=pan, in_=w_dram.ap()[m])
                        z = zp.tile([128, 512], WDT, tag=f"z{w_dram.name}{m}",
                                    name=f"z{w_dram.name}{m}_{c}")
                        zs.append(z)
                        ps = mmp.tile([128, 512], F32, tag="mm", name=f"za{m}_{c}")
                        for k in range(KX):
                            nc.tensor.matmul(ps, pan[:, k, :], xt[k],
                                             start=(k == 0), stop=(k == KX - 1))
                        nc.scalar.activation(z, ps, AF.Copy)
                        if m < nsq:
                            sq = sqp.tile([128, 512], WDT, tag="sq",
                                          name=f"sq{m}_{c}")
                            nc.scalar.activation(sq, ps, AF.Square)
                            nc.tensor.matmul(sq_ps, ones, sq,
                                             start=(m == 0), stop=(m == nsq - 1),
                                             skip_group_check=True)

                def stats_apply(zs, nf, mu_src, nm):
                    # returns nothing; applies (z - mu) * r in place
                    mu_row = rowp.tile([1, 512], WDT, tag="row", name=f"mu{nm}{c}")
                    nc.scalar.activation(mu_row, mu_src, AF.Copy)
                    mu2 = rowp.tile([1, 512], F32, tag="row", name=f"mu2{nm}{c}")
                    nc.vector.tensor_mul(mu2, mu_row, mu_row)
                    var = rowp.tile([1, 512], F32, tag="row", name=f"var{nm}{c}")
                    nc.vector.scalar_tensor_tensor(var, sq_kv if nm == "k" else sq_q,
                                                   1.0 / nf, mu2,
                                                   op0=AluOpType.mult,
                                                   op1=AluOpType.subtract)
                    std = rowp.tile([1, 512], F32, tag="row", name=f"std{nm}{c}")
                    nc.scalar.activation(std, var, AF.Sqrt, bias=eps_t)
                    r = rowp.tile([1, 512], WDT, tag="row", name=f"r{nm}{c}")
                    with nc.allow_low_precision(reason="f32r is 4-byte fp32"):
                        nc.vector.reciprocal(r, std)
                    mur = rowp.tile([1, 512], WDT, tag="row", name=f"mur{nm}{c}")
                    nc.vector.tensor_mul(mur, mu_row, r)
                    rb = lnbp.tile([128, 512], F32, tag="lnb", name=f"rb{nm}{c}")
                    murb = lnbp.tile([128, 512], F32, tag="lnb", name=f"murb{nm}{c}")
                    for row, dst in ((r, rb), (mur, murb)):
                        ps = mmp.tile([128, 512], F32, tag="mm", name=f"bc{nm}{c}")
                        nc.tensor.matmul(ps, brow, row, start=True, stop=True)
                        nc.scalar.activation(dst, ps, AF.Copy)
                    for z in zs:
                        nc.vector.tensor_mul(z, z, rb)
                        nc.vector.tensor_sub(z, z, murb)


                zkv = []
                sq_kv = arp.tile([1, 512], F32, tag="ar", name=f"sqkv{c}")
                aproj(5, wkva_d, sq_kv, KKV, zkv, pre=pan0)
                kbpans = []
                for m in range(HPC):
                    kbp = wp.tile([128, KKV, 128], WDT, tag="wsm", bufs=4,
                                  name=f"pkb{m}_{c}")
                    nc.gpsimd.dma_start(out=kbp, in_=wkbk_d.ap()[m])
                    kbpans.append(kbp)
                stats_apply(zkv[:KKV], KL, zkv[4][96:97, :], "k")
                zq = []
                sq_q = arp.tile([1, 512], F32, tag="ar", name=f"sqq{c}")
                aproj(KQ, wqa_d, sq_q, KQ, zq)

                # kv_b: k_nope columns for this chunk
                for m in range(HPC):
                    pan = kbpans[m]
                    ps = mmp.tile([128, 512], F32, tag="mm", name=f"kb{m}_{c}")
                    for k in range(KKV):
                        nc.tensor.matmul(ps, pan[:, k, :], zkv[k],
                                         start=(k == 0), stop=(k == KKV - 1))
                    nc.vector.tensor_scalar_add(knope[m][:, ch], ps, ckv[:, m:m + 1])

                # V (token-major) for this chunk's 4 s-tiles
                wkbv = wp.tile([128, KKV, HPC * DV], WDT, tag="w",
                               name=f"wkbv{c}")
                nc.gpsimd.dma_start(out=wkbv, in_=wkbv_d.ap())
                for si in range(4):
                    st = c * 4 + si
                    ps = mmp.tile([128, 512], F32, tag="mm", name=f"v{st}")
                    for k in range(KKV):
                        nc.tensor.matmul(ps, zkv[k][:, si * 128:(si + 1) * 128],
                                         wkbv[:, k, :],
                                         start=(k == 0), stop=(k == KKV - 1))
                    nc.scalar.activation(vt[st], ps, AF.Copy)

                # k_rope: duplicate to both 64-halves and rotate
                kraw = zkv[4]
                d_ps = mmp.tile([128, 512], F32, tag="mm", name=f"kd{c}")
                nc.tensor.matmul(d_ps, pdup, kraw[0:64, :], start=True, stop=True)
                dsw_ps = mmp.tile([128, 512], F32, tag="mm", name=f"kds{c}")
                nc.tensor.matmul(dsw_ps, pdupsw, kraw[0:64, :], start=True, stop=True)
                t2 = sqp.tile([128, 512], WDT, tag="sq", name=f"kt2{c}")
                nc.vector.tensor_mul(t2, dsw_ps, s_t)
                t3 = sqp.tile([128, 512], WDT, tag="sq", name=f"kt3{c}")
                nc.vector.tensor_mul(t3, d_ps, c_t)
                nc.vector.tensor_add(krope[:, ch], t3, t2)

                # q path
                stats_apply(zq, QL, zkv[4][64:65, :], "q")
                for m in range(MQB):
                    pan = wp.tile([128, KQ, 128], WDT, tag="w", name=f"pqb{m}_{c}")
                    nc.sync.dma_start(out=pan, in_=wqb_d.ap()[m])
                    ps = mmp.tile([128, 512], F32, tag="mm", name=f"qb{m}_{c}")
                    for k in range(KQ):
                        nc.tensor.matmul(ps, pan[:, k, :], zq[k],
                                         start=(k == 0), stop=(k == KQ - 1))
                    nc.vector.tensor_scalar_add(qfull[m][:, ch], ps, cq[:, m:m + 1])

                # rope on the two q pair tiles (in place)
                for i in range(2):
                    src = qfull[HPC + i]
                    sw_ps = mmp.tile([128, 512], F32, tag="mm", name=f"qsw{i}_{c}")
                    nc.tensor.matmul(sw_ps, pswap, src[:, ch], start=True, stop=True)
                    t2 = sqp.tile([128, 512], WDT, tag="sq", name=f"qt2{i}{c}")
                    nc.vector.tensor_mul(t2, sw_ps, s_t)
                    t3 = sqp.tile([128, 512], WDT, tag="sq", name=f"qt3{i}{c}")
                    nc.vector.tensor_mul(t3, src[:, ch], c_t)
                    nc.vector.tensor_add(src[:, ch], t3, t2)

            # ---- attention (k-major, causal): q-chunk outer, head inner.
            # The divide/finalize of unit n is deferred until after unit n+1's
            # matmul stream so the PE (in-order) never waits on the DVE
            # reciprocal at a unit boundary.
            pending = None
            for c in range(2):
                for h in range(HPC):
                    base = 64 * (h % 2)
                    qr = qfull[HPC + h // 2]
                    num = nump.tile([128, 512], F32, tag="num", name=f"num{h}_{c}")
                    den = denp.tile([1, 512], F32, tag="den", name=f"den{h}_{c}")
                    last_ki = (c * 512 + 511) // 128
                    for ki in range(last_ki + 1):
                        q0 = ki * 128
                        lo, hi = max(q0, c * 512), (c + 1) * 512
                        w = hi - lo
                        ps = mmp.tile([128, 512], F32, tag="mm",
                                      name=f"sc{h}_{ki}_{c}")
                        nc.tensor.matmul(ps[:, 0:w],
                                         knope[h][:, q0:q0 + 128],
                                         qfull[h][:, lo:hi], start=True, stop=False)
                        nc.tensor.matmul(ps[:, 0:w],
                                         krope[base:base + 64, q0:q0 + 128],
                                         qr[base:base + 64, lo:hi],
                                         start=False, stop=True)
                        p = ptp.tile([128, 512], WDT, tag="p",
                                     name=f"p{h}_{ki}_{c}")
                        nc.scalar.activation(p[:, 0:w], ps[:, 0:w], AF.Exp,
                                             scale=SCALE)
                        if lo == q0:  # diagonal block: causal triangle
                            nc.vector.tensor_mul(p[:, 0:128], p[:, 0:128], tri)
                        nc.tensor.matmul(num[:, lo - c * 512:512],
                                         vt[ki][:, h * 128:(h + 1) * 128],
                                         p[:, 0:w],
                                         start=(ki == 0), stop=(ki == last_ki),
                                         skip_group_check=True)
                        nc.tensor.matmul(den[:, lo - c * 512:512],
                                         ones, p[:, 0:w],
                                         start=(ki == 0), stop=(ki == last_ki),
                                         skip_group_check=True)

                    def finalize(h=h, c=c, num=num, den=den):
                        rec = rowp.tile([1, 512], WDT, tag="row",
                                        name=f"rec{h}_{c}")
                        with nc.allow_low_precision(reason="f32r is fp32 bits"):
                            nc.vector.reciprocal(rec, den)
                        rb_ps = mmp.tile([128, 512], F32, tag="mm",
                                         name=f"rb{h}_{c}")
                        nc.tensor.matmul(rb_ps, brow, rec, start=True, stop=True)
                        rb_sb = sqp.tile([128, 512], F32, tag="sq",
                                         name=f"rbs{h}_{c}")
                        nc.scalar.activation(rb_sb, rb_ps, AF.Copy)
                        nc.vector.tensor_mul(attn[h][:, CH[c]], num, rb_sb)
                        nc.vector.tensor_scalar_add(attn[h][:, CH[c]],
                                                    attn[h][:, CH[c]],
                                                    bvc[:, h:h + 1])

                    if pending is not None:
                        pending()
                    pending = finalize

            # ---- o_proj partials (last attention finalize is slotted after
            # the first chunk's matmuls so the PE never waits on it) ----
            for m in range(MO):
                pan = wp.tile([128, HPC, 128], WDT, tag="wsm", bufs=4,
                              name=f"po{m}")
                nc.sync.dma_start(out=pan, in_=wo_d.ap()[m])
                for c in range(2):
                    ps = mmp.tile([128, 512], F32, tag="mm", name=f"op{m}_{c}")
                    for k in range(HPC):
                        nc.tensor.matmul(ps, pan[:, k, :], attn[k][:, CH[c]],
                                         start=(k == 0), stop=(k == HPC - 1))
                    if pending is not None and m == 0 and c == 0:
                        pending()
                        pending = None
                    ot = lnbp.tile([128, 512], F32, tag="lnb", name=f"o{m}_{c}")
                    if m % 2 == 0:
                        nc.scalar.activation(ot, ps, AF.Copy)
                        nc.sync.dma_start(
                            out=o_d.ap()[m * 128:(m + 1) * 128, CH[c]], in_=ot)
                    else:
                        nc.vector.tensor_copy(ot, ps)
                        nc.scalar.dma_start(
                            out=o_d.ap()[m * 128:(m + 1) * 128, CH[c]], in_=ot)

    nc.compile()
    return nc


def _host_prep(x, w_qkv_a, q_ln_g, q_ln_b, w_q_b, w_kv_a, kv_ln_g, kv_ln_b,
               w_kv_b, w_o, freqs_cos, freqs_sin):
    f32 = np.float32
    x = np.asarray(x, f32)
    w_qkv_a = np.asarray(w_qkv_a, f32)
    w_q_b = np.asarray(w_q_b, f32)
    w_kv_a = np.asarray(w_kv_a, f32)
    w_kv_b = np.asarray(w_kv_b, f32)
    w_o = np.asarray(w_o, f32)
    q_ln_g = np.asarray(q_ln_g, f32)
    q_ln_b = np.asarray(q_ln_b, f32)
    kv_ln_g = np.asarray(kv_ln_g, f32)
    kv_ln_b = np.asarray(kv_ln_b, f32)
    cos = np.asarray(freqs_cos, f32)  # [S, 32]
    sin = np.asarray(freqs_sin, f32)

    # interleaved rope dims -> half-split permutation (even dims then odd)
    rp = np.concatenate([np.arange(0, DR, 2), np.arange(1, DR, 2)])

    wqa = w_qkv_a[:, :QL]                                  # [2048, 1536]
    # kv a-proj augmented: [w_kv_a | rope perm | mu_q col | mu_kv col | pad]
    wkva = np.zeros((HID, 5 * 128), f32)
    wkva[:, :KL] = w_kv_a[:, :KL]
    wkva[:, KL:KL + DR] = w_kv_a[:, KL:][:, rp]
    # mu columns at 32-aligned in-tile partitions of M-tile 4 (rows 64, 96)
    wkva[:, KL + 64] = wqa.mean(axis=1)
    wkva[:, KL + 96] = w_kv_a[:, :KL].mean(axis=1)

    def panels(w, kt, mt):
        # [K, M] -> [mt, 128, kt, 128]: partition-major so DMA rows are
        # contiguous kt*512-byte runs
        return np.ascontiguousarray(
            w.reshape(kt, 128, mt, 128).transpose(2, 1, 0, 3))

    # q_b weights: gamma-folded, per-core head slice, col order:
    # [h0n h1n h2n h3n | h0r h1r | h2r h3r], rope dims half-split
    wqb_g = (w_q_b * q_ln_g[:, None]).reshape(QL, H, DN + DR)
    cq_full = (q_ln_b @ w_q_b).reshape(H, DN + DR)
    wkb_g = (w_kv_b * kv_ln_g[:, None]).reshape(KL, H, DN + DV)
    ckv_full = (kv_ln_b @ w_kv_b).reshape(H, DN + DV)

    c128 = np.tile(cos.T, (4, 1)).astype(f32)                    # [128, S]
    s128 = np.tile(np.vstack([-sin.T, sin.T]), (2, 1)).astype(f32)
    tri = np.triu(np.ones((128, 128), f32))                      # keep q>=k
    ones_col = np.ones((128, 1), f32)
    brow = np.ones((1, 128), f32)
    pswap = np.zeros((128, 128), f32)
    for m in range(128):
        pswap[m ^ 32, m] = 1.0
    pdup = np.zeros((64, 128), f32)
    pdupsw = np.zeros((64, 128), f32)
    for m in range(128):
        pdup[m % 64, m] = 1.0
        pdupsw[(m % 64) ^ 32, m] = 1.0

    in_maps = []
    for core in range(NCORES):
        b = core // TP
        h0 = (core % TP) * HPC
        heads = list(range(h0, h0 + HPC))

        wqb_c = np.zeros((QL, MQB * 128), f32)
        cq_c = np.zeros(MQB * 128, f32)
        for i, h in enumerate(heads):
            wqb_c[:, i * 128:(i + 1) * 128] = wqb_g[:, h, :DN]
            cq_c[i * 128:(i + 1) * 128] = cq_full[h, :DN]
            off = HPC * 128 + i * 64
            wqb_c[:, off:off + 64] = wqb_g[:, h, DN:][:, rp]
            cq_c[off:off + 64] = cq_full[h, DN:][rp]

        wkbk_c = np.zeros((KL, HPC * 128), f32)
        ckv_c = np.zeros(HPC * 128, f32)
        wkbv_c = np.zeros((KL, HPC * 128), f32)
        bv_c = np.zeros(HPC * 128, f32)
        for i, h in enumerate(heads):
            wkbk_c[:, i * 128:(i + 1) * 128] = wkb_g[:, h, :DN]
            ckv_c[i * 128:(i + 1) * 128] = ckv_full[h, :DN]
            wkbv_c[:, i * 128:(i + 1) * 128] = wkb_g[:, h, DN:]
            bv_c[i * 128:(i + 1) * 128] = ckv_full[h, DN:]

        wo_c = w_o.reshape(H, DV, HID)[heads].reshape(HPC * DV, HID)

        wt = np.float32
        if USE_BF16:
            import ml_dtypes
            wt = ml_dtypes.bfloat16
        in_maps.append({
            "xT": np.ascontiguousarray(x[b].T).reshape(KX, 128, S).astype(wt),
            "wqa": panels(wqa, KX, KQ).astype(wt),
            "wkva": panels(wkva, KX, 5).astype(wt),
            "wqb": panels(wqb_c, KQ, MQB).astype(wt),
            "wkbk": panels(wkbk_c, KKV, HPC).astype(wt),
            "wkbv": np.ascontiguousarray(wkbv_c.reshape(KKV, 128, HPC * 128).transpose(1, 0, 2)).astype(wt),
            "wo": panels(wo_c, HPC, MO).astype(wt),
            "c128": c128, "s128": s128, "tri": tri,
            "ones": ones_col.astype(wt), "brow": brow.astype(wt),
            "pswap": pswap.astype(wt), "pdup": pdup.astype(wt), "pdupsw": pdupsw.astype(wt),
            "cq": np.ascontiguousarray(cq_c.reshape(MQB, 128).T),
            "ckv": np.ascontiguousarray(ckv_c.reshape(HPC, 128).T),
            "bvc": np.ascontiguousarray(bv_c.reshape(HPC, 128).T),
        })
    return in_maps


def kernel(**inputs):
    global _COMPILED
    if _COMPILED is None:
        _COMPILED = _build()
    nc = _COMPILED
    in_maps = _host_prep(**inputs)
    from concourse.bass_utils import run_bass_kernel_spmd
    res = run_bass_kernel_spmd(nc, in_maps, core_ids=list(range(NCORES)),
                               trace=TRACE)
    kernel.last_results = res
    out = np.empty((B, S, HID), np.float32)
    for b in range(B):
        acc = res.results[b * TP][ "o_part"].astype(np.float64)
        for t in range(1, TP):
            acc += res.results[b * TP + t]["o_part"]
        out[b] = acc.T.astype(np.float32)
    return out

